# revision 9
# baseline (speedup 1.0000x reference)
"""GCMC graph-conv kernel for Trainium2, 8-core SPMD.

out = ci * segment_sum((weight[node_ids] * cj)[src_idx], dst_idx)

Strategy (edge sharding by balanced dst bins):
  - cj is folded into the weight table on the host (wtab = weight[node_ids]*cj)
  - the 100k dst rows are packed into 8*98 bins of <=128 rows with a
    variance-balancing pass so that every (bin, src-chunk) segment holds
    ~1020 edges; segments are clamped at 1024 edges and the excess edges of a
    few rows spill to a per-core overflow tile (rows split across two tiles
    have their partial outputs added on the host)
  - messages are fetched from the wtab with SWDGE dma_gather (int16 indices,
    4 chunks of 25000 rows); gathers are batched per (4-tile group, chunk)
    window to amortize the ~1us fixed SWDGE cost per call
  - segment-sum via one-hot matmul: DVE builds onehot[p, d] = (iota[d] ==
    dst_local[p]) per 128-slot block, TensorE accumulates psum[d, :] +=
    onehot.T @ msg
  - ACT copies psum*ci into a resident [128, 99, 64] output tile; 8 batched
    partition-major DMAs write it out (big descriptors, full DMA bandwidth)
"""
import sys, os
sys.path.insert(0, '/opt/trn_rl_repo')

import numpy as np

N_NODES = 100000
OUT_DIM = 64
N_CORES = 8
N_TILES = 98                  # main tiles per core
N_ALL_TILES = N_TILES + 1     # + overflow tile
NB = N_CORES * N_TILES        # 784 bins
N_CHUNKS = 4
CHUNK = N_NODES // N_CHUNKS   # 25000
SEG_CAP = 1024                # clamp: max edges per (bin, chunk)
GROUP = 4                     # tiles per gather window
PAD_SENTINEL = 999.0


def _round_up(x, m):
    return (x + m - 1) // m * m


def _balance_bins(v):
    """Assign each dst row to one of 784 bins (<=128 rows each) so per-bin
    per-chunk edge counts are near-equal.  v: [N_NODES, 4] chunk-degree."""
    tot = v.sum(1)
    target = v.sum(0) / NB
    order = np.argsort(-tot, kind="stable")
    bin_of = np.empty(N_NODES, np.int32)
    idx = np.arange(N_NODES)
    wave, pos = idx // NB, idx % NB
    snake = np.where(wave % 2 == 0, pos, NB - 1 - pos)
    bin_of[order] = snake.astype(np.int32)

    L = np.stack([np.bincount(bin_of, weights=v[:, k], minlength=NB)
                  for k in range(4)], 1).astype(np.int64)
    for it in range(160):
        cc = it % 4
        D = L - target
        devc = D[:, cc]
        hi = np.argsort(-devc)[: NB // 4]
        lo = np.argsort(devc)[: NB // 4]
        order2 = np.argsort(bin_of, kind="stable")
        bstart = np.searchsorted(bin_of[order2], np.arange(NB))
        bend = np.searchsorted(bin_of[order2], np.arange(NB) + 1)
        for h, l in zip(hi, lo):
            rh = order2[bstart[h]:bend[h]]
            rl = order2[bstart[l]:bend[l]]
            a = rh[np.argmax(v[rh, cc])]
            b = rl[np.argmin(v[rl, cc])]
            delta = v[a] - v[b]
            dh = L[h] - target
            dl = L[l] - target
            gain = ((dh**2 + dl**2) - ((dh - delta)**2 + (dl + delta)**2)).sum()
            if gain > 0:
                bin_of[a], bin_of[b] = l, h
                L[h] -= delta
                L[l] += delta
    return bin_of, L


def _host_prep(src, dst):
    """Returns the full slot layout shared by all cores plus per-core
    metadata (gather indices, dst values, row maps)."""
    c_of_edge = (src // CHUNK).astype(np.int64)
    v = np.zeros((N_NODES, 4), np.int64)
    np.add.at(v, (dst, c_of_edge), 1)

    bin_of, L = _balance_bins(v)

    # rank-pair bins across cores: sorted profile groups of 8 become one tile
    key = np.lexsort((L[:, 3], L[:, 2], L[:, 1], L[:, 0]))
    tile_of_bin = np.empty(NB, np.int32)
    core_of_bin = np.empty(NB, np.int32)
    for j in range(N_TILES):
        grp = key[8 * j: 8 * j + 8]
        tile_of_bin[grp] = j
        core_of_bin[grp] = np.arange(8)

    core_of_row = core_of_bin[bin_of]
    tile_of_row = tile_of_bin[bin_of]

    # clamp + spill: per (core, tile, chunk) over SEG_CAP, move whole
    # (row, chunk) edge sets of the heaviest rows to the overflow tile
    cnt = np.zeros((N_CORES, N_TILES, 4), np.int64)
    np.add.at(cnt, (core_of_row[dst], tile_of_row[dst], c_of_edge), 1)
    spill_mask = np.zeros((N_NODES, 4), bool)
    spill_rows = [[] for _ in range(N_CORES)]
    rows_by_bin = {}
    for k in range(N_CORES):
        for t in range(N_TILES):
            for cc in range(4):
                over = cnt[k, t, cc] - SEG_CAP
                if over <= 0:
                    continue
                b = np.where((core_of_bin == k) & (tile_of_bin == t))[0][0]
                if b not in rows_by_bin:
                    rows_by_bin[b] = np.where(bin_of == b)[0]
                rows = rows_by_bin[b]
                vv = v[rows, cc] * ~spill_mask[rows, cc]
                sel = np.argsort(-vv)
                for s in sel:
                    if over <= 0:
                        break
                    r = rows[s]
                    if vv[s] == 0:
                        break
                    spill_mask[r, cc] = True
                    over -= vv[s]
                    if r not in spill_rows[k]:
                        spill_rows[k].append(r)
    for k in range(N_CORES):
        assert len(spill_rows[k]) <= 128, (k, len(spill_rows[k]))

    # slot assignment within tiles
    slot_of_row = np.full(N_NODES, -1, np.int32)
    row_maps = np.full((N_CORES, N_ALL_TILES, 128), -1, np.int64)
    for b in range(NB):
        rows = rows_by_bin.get(b)
        if rows is None:
            rows = np.where(bin_of == b)[0]
        k, t = core_of_bin[b], tile_of_bin[b]
        slot_of_row[rows] = np.arange(len(rows))
        row_maps[k, t, :len(rows)] = rows
    spill_slot = np.full(N_NODES, -1, np.int32)
    for k in range(N_CORES):
        rs = np.array(spill_rows[k], np.int64)
        if len(rs):
            spill_slot[rs] = np.arange(len(rs))
            row_maps[k, N_ALL_TILES - 1, :len(rs)] = rs

    # per-edge effective tile/slot
    e_core = core_of_row[dst]
    e_spill = spill_mask[dst, c_of_edge]
    e_tile = np.where(e_spill, N_ALL_TILES - 1, tile_of_row[dst])
    e_slot = np.where(e_spill, spill_slot[dst], slot_of_row[dst])
    assert (e_slot >= 0).all()

    # clamped per-core counts and the shared envelope
    cnt_all = np.zeros((N_CORES, N_ALL_TILES, 4), np.int64)
    np.add.at(cnt_all, (e_core, e_tile, c_of_edge), 1)
    assert (cnt_all[:, :N_TILES] <= SEG_CAP).all()
    env = _round_up(cnt_all.max(axis=0), 128)          # [99, 4]

    # window layout: groups of GROUP tiles (overflow joins the last group);
    # slot order is (group, chunk, tile, src)
    groups = [list(range(g, min(g + GROUP, N_ALL_TILES)))
              for g in range(0, N_ALL_TILES, GROUP)]
    seg_off = np.zeros((N_ALL_TILES, 4), np.int64)
    win_off = {}
    off = 0
    win_list = []
    for gi, g in enumerate(groups):
        for cc in range(4):
            w = int(sum(env[t, cc] for t in g))
            win_off[(gi, cc)] = off
            if w > 0:
                win_list.append((gi, cc, off, w))
            for t in g:
                seg_off[t, cc] = off
                off += int(env[t, cc])
    total = off
    assert total % 128 == 0

    # per-edge slot position: order by (core, group, chunk, tile, src)
    e_group = e_tile // GROUP
    order = np.lexsort((src, e_tile, c_of_edge, e_group, e_core))
    seg_id = (e_core * N_ALL_TILES + e_tile) * 4 + c_of_edge
    so = seg_id[order]
    change = np.concatenate([[True], so[1:] != so[:-1]])
    run_start = np.where(change)[0]
    within = np.arange(len(so)) - run_start[np.cumsum(change) - 1]
    slot_global = seg_off[e_tile[order], c_of_edge[order]] + within

    idx_all, dv_all = [], []
    e_core_o = e_core[order]
    src_o = src[order]
    c_o = c_of_edge[order]
    slot_o = e_slot[order]
    for k in range(N_CORES):
        m = e_core_o == k
        idx_flat = np.zeros(total, np.int16)
        idx_flat[slot_global[m]] = (src_o[m] - c_o[m] * CHUNK).astype(np.int16)
        dv_flat = np.full(total, 255, np.uint8)
        dv_flat[slot_global[m]] = slot_o[m].astype(np.uint8)
        idx_all.append(idx_flat.reshape(total // 16, 16).T.copy())
        dv_all.append(dv_flat.reshape(total // 128, 128).T.copy())
    return env, seg_off, win_list, groups, total, idx_all, dv_all, row_maps


def _build_program(env, seg_off, win_list, groups, total):
    import concourse.bass as bass
    import concourse.bacc as bacc
    import concourse.mybir as mybir
    import concourse.tile as tile

    f32 = mybir.dt.float32
    nc = bacc.Bacc("TRN2", target_bir_lowering=False, debug=False,
                   num_devices=N_CORES)
    w_d = nc.dram_tensor("w", [N_NODES, OUT_DIM], f32, kind="ExternalInput").ap()
    ci_d = nc.dram_tensor("ci", [128, N_ALL_TILES], f32, kind="ExternalInput").ap()
    iota_d = nc.dram_tensor("iota", [128, 128], f32, kind="ExternalInput").ap()
    idx_d = nc.dram_tensor("idx", [16, total // 16], mybir.dt.int16,
                           kind="ExternalInput").ap()
    dv_d = nc.dram_tensor("dv", [128, total // 128], mybir.dt.uint8,
                          kind="ExternalInput").ap()
    out_d = nc.dram_tensor("out", [128, N_ALL_TILES, OUT_DIM], f32,
                           kind="ExternalOutput").ap()

    blocks = env.sum(axis=1) // 128          # per-tile block count
    assert (blocks[:N_TILES] > 0).all()

    with tile.TileContext(nc) as tc:
        with (
            tc.tile_pool(name="const", bufs=1) as constp,
            tc.tile_pool(name="msg", bufs=8) as msgp,
            tc.tile_pool(name="oh", bufs=8) as ohp,
            tc.tile_pool(name="ps", bufs=4, space="PSUM") as psp,
        ):
            ci_t = constp.tile([128, N_ALL_TILES], f32)
            io_t = constp.tile([128, 128], f32)
            idx_t = constp.tile([128, total // 16], mybir.dt.int16)
            dv_t = constp.tile([128, total // 128], mybir.dt.uint8)
            out_t = constp.tile([128, N_ALL_TILES, OUT_DIM], f32)
            nc.sync.dma_start(ci_t[:], ci_d[:])
            nc.sync.dma_start(io_t[:], iota_d[:])
            n_up = 16
            cols16 = _round_up(total // 16 // n_up, 16)
            cols128 = _round_up(total // 128 // n_up, 4)
            for u in range(n_up):
                a, b = u * cols16, min((u + 1) * cols16, total // 16)
                if a < b:
                    # upload once into partitions 0-15, replicate on DVE for
                    # the other 7 SWDGE core stripes (int16-exact add 0)
                    nc.sync.dma_start(idx_t[0:16, a:b], idx_d[:, a:b])
                    for r in range(1, 8):
                        nc.vector.tensor_scalar_add(
                            idx_t[16 * r:16 * r + 16, a:b], idx_t[0:16, a:b], 0)
                a, b = u * cols128, min((u + 1) * cols128, total // 128)
                if a < b:
                    nc.sync.dma_start(dv_t[:, a:b], dv_d[:, a:b])

            # window gathers, keyed (group, chunk)
            win_tiles = {}
            for (gi, cc, off, w) in win_list:
                msg = msgp.tile([128, w // 128, OUT_DIM], f32, tag="msg")
                nc.gpsimd.dma_gather(
                    msg[:], w_d[cc * CHUNK:cc * CHUNK + CHUNK, :],
                    idx_t[:, off // 16:(off + w) // 16],
                    w, w, OUT_DIM,
                    single_packet=(w <= 1024),
                )
                win_tiles[(gi, cc)] = (msg, off)

            flushed = 0
            for gi, g in enumerate(groups):
                for t in g:
                    n_blk = int(blocks[t])
                    if n_blk == 0:
                        nc.vector.memset(out_t[:, t, :], 0.0)
                        continue
                    ps = psp.tile([128, OUT_DIM], f32)
                    bi = 0
                    for cc in range(4):
                        e_tc = int(env[t, cc])
                        if e_tc == 0:
                            continue
                        msg, woff = win_tiles[(gi, cc)]
                        col0 = (int(seg_off[t, cc]) - woff) // 128
                        g0 = int(seg_off[t, cc]) // 128
                        for b in range(e_tc // 128):
                            oh = ohp.tile([128, 128], f32, tag="oh")
                            nc.vector.tensor_scalar(
                                oh[:], io_t[:], dv_t[:, g0 + b:g0 + b + 1],
                                None, mybir.AluOpType.is_equal)
                            nc.tensor.matmul(ps[:], oh[:], msg[:, col0 + b, :],
                                             start=(bi == 0),
                                             stop=(bi == n_blk - 1))
                            bi += 1
                    nc.scalar.activation(out_t[:, t, :], ps[:],
                                         mybir.ActivationFunctionType.Copy,
                                         scale=ci_t[:, t:t + 1])
                # flush finished tiles in batches of ~13
                done = g[-1] + 1
                if done - flushed >= 13 or gi == len(groups) - 1:
                    nc.sync.dma_start(out_d[:, flushed:done, :],
                                      out_t[:, flushed:done, :])
                    flushed = done

    nc.compile()
    return nc


def prepare(node_ids, src_idx, dst_idx, cj, ci, weight):
    """Host prep + program build. Returns (nc, in_maps, postprocess)."""
    import time
    _t0 = time.time()

    node_ids = np.asarray(node_ids)
    src = np.asarray(src_idx).astype(np.int64)
    dst = np.asarray(dst_idx).astype(np.int64)
    cj = np.asarray(cj, dtype=np.float32).reshape(-1)
    ci = np.asarray(ci, dtype=np.float32).reshape(-1)
    weight = np.asarray(weight, dtype=np.float32)

    # feat rows are weight[node_ids]; with the arange fill this is identity
    if not np.array_equal(node_ids, np.arange(N_NODES, dtype=node_ids.dtype)):
        weight = weight[node_ids]
    wtab = np.ascontiguousarray(weight * cj[:, None])

    iota = np.tile(np.arange(128, dtype=np.float32), (128, 1))

    env, seg_off, win_list, groups, total, idx_all, dv_all, row_maps = \
        _host_prep(src, dst)
    print(f"[kernel] host prep: {time.time()-_t0:.1f}s (total slots {total})",
          flush=True)
    _t1 = time.time()
    nc = _build_program(env, seg_off, win_list, groups, total)
    print(f"[kernel] build+schedule+compile-to-bir: {time.time()-_t1:.1f}s",
          flush=True)

    in_maps = []
    for k in range(N_CORES):
        ci_w = np.zeros((128, N_ALL_TILES), np.float32)
        for t in range(N_ALL_TILES):
            rows = row_maps[k, t]
            m = rows >= 0
            ci_w[m, t] = ci[rows[m]]
        in_maps.append({
            "w": wtab, "ci": ci_w, "iota": iota,
            "idx": idx_all[k], "dv": dv_all[k],
        })

    def post(results):
        full = np.zeros((N_NODES, OUT_DIM), np.float32)
        for k in range(N_CORES):
            dat = results[k]["out"]                    # [128, 99, 64]
            for t in range(N_ALL_TILES):
                rows = row_maps[k, t]
                m = rows >= 0
                if t < N_TILES:
                    full[rows[m]] = dat[m, t]
                else:
                    full[rows[m]] += dat[m, t]
        return full

    return nc, in_maps, post


def kernel(node_ids, src_idx, dst_idx, cj, ci, weight):
    import time
    from concourse.bass_utils import run_bass_kernel_spmd
    nc, in_maps, post = prepare(node_ids, src_idx, dst_idx, cj, ci, weight)
    _t2 = time.time()
    res = run_bass_kernel_spmd(nc, in_maps, core_ids=list(range(N_CORES)))
    print(f"[kernel] neff compile+exec: {time.time()-_t2:.1f}s", flush=True)
    return post(res.results)
